# revision 1
# baseline (speedup 1.0000x reference)
"""DA-RNN style encoder (LSTM + input attention) on 8 Trainium2 cores.

Problem: nn_Encoder_63024350101963
  B=2048, T-1=31 steps, D=128 input feats, H=128 hidden.

Key algebraic fact exploited: in the reference,
    score = (h @ w_h + c @ w_c + b)[:, None] + x_score
the recurrent term is constant along the softmax axis, and softmax is
shift-invariant, so
    attn = softmax(x_score)      (time-constant, recurrence-independent)
Therefore weighted = attn[:,None,:] * x  is a pure elementwise op and only
the LSTM cell recurrence is serial.

Device layout: feature-on-partitions, batch-on-free ("transposed") all the
way through; the host passes x pre-transposed [D, T, B_local] and
re-transposes the outputs, so the device never transposes the big tensors.
All transcendentals are sigmoid-only: tanh(z) = 2*sigmoid(2z) - 1, with the
factor 2 folded into the g-gate weights and the affine fix fused into the
DVE affine_mul_reduce op.

PSUM layout: two ping-pong tiles [128, 2048] (4 banks each); bank c holds
gate-chunk c for two consecutive steps x two batch subtiles, so the bias
and W_ih matmuls run at N=512 with one weight load per two steps.  Only
the W_hh matmuls (N=128) are per-step, as the recurrence requires.

Sharding: data-parallel over batch, 8 cores x 256 rows, weights replicated.
"""

import numpy as np

T = 31          # time steps (T_ref - 1)
D = 128         # input feature dim
H = 128         # hidden dim
G = 4 * H       # gate rows
NCORES = 8
B = 2048
BL = B // NCORES  # 256 batch rows per core
BS = 128          # batch sub-tile (2 per core)
NS = BL // BS
F32R = True       # use the fast fp32r PE path for matmuls (validated on HW)

_CACHE = {}


def _build_program(loop_n=0):
    from contextlib import ExitStack

    import concourse.bacc as bacc
    import concourse.mybir as mybir
    import concourse.tile as tile

    dt = mybir.dt.float32
    AF = mybir.ActivationFunctionType

    nc = bacc.Bacc("TRN2", target_bir_lowering=False, debug=False)

    xt_d = nc.dram_tensor("xt", [D, T, BL], dt, kind="ExternalInput").ap()
    wxb_d = nc.dram_tensor("wxb", [D, T], dt, kind="ExternalInput").ap()
    wih_d = nc.dram_tensor("wih", [D, G], dt, kind="ExternalInput").ap()
    whh_d = nc.dram_tensor("whh", [H, G], dt, kind="ExternalInput").ap()
    bias_d = nc.dram_tensor("bias", [2, G], dt, kind="ExternalInput").ap()
    ident_d = nc.dram_tensor("ident", [D, D], dt, kind="ExternalInput").ap()
    ones_d = nc.dram_tensor("ones", [2, 2 * BL], dt, kind="ExternalInput").ap()

    wt_d = nc.dram_tensor("wt_out", [T, D, BL], dt, kind="ExternalOutput").ap()
    enc_d = nc.dram_tensor("enc_out", [T, H, BL], dt, kind="ExternalOutput").ap()

    with ExitStack() as ctx:
        tc = ctx.enter_context(tile.TileContext(nc))

        def body():
            _emit(nc, tc, ctx, mybir, dt, AF,
                  xt_d, wxb_d, wih_d, whh_d, bias_d, ident_d, ones_d,
                  wt_d, enc_d)

        if loop_n:
            with tc.For_i(0, loop_n, 1):
                body()
        else:
            body()

    nc.compile()
    return nc


def _emit(nc, tc, ctx, mybir, dt, AF,
          xt_d, wxb_d, wih_d, whh_d, bias_d, ident_d, ones_d, wt_d, enc_d):
    from contextlib import ExitStack
    import concourse.bass as bass

    def rr(ap):
        return ap.bitcast(mybir.dt.float32r) if F32R else ap

    big = ctx.enter_context(tc.tile_pool(name="big", bufs=1))

    # ---- persistent SBUF tensors ----
    xt_s = big.tile([D, T * BL], dt, tag="xt")
    wid_s = big.tile([D, T * D], dt, tag="wid")
    wxb_s = big.tile([D, T], dt, tag="wxb")
    wxt_s = big.tile([D, T * BL], dt, tag="wxt")
    wxr_s = big.tile([D, T * BL], dt, tag="wxr")
    wih_s = big.tile([D, G], dt, tag="wih")
    whh_s = big.tile([H, G], dt, tag="whh")
    bias_s = big.tile([2, G], dt, tag="bias")
    ident_s = big.tile([D, D], dt, tag="ident")
    ones_s = big.tile([2, 2 * BL], dt, tag="ones")
    zro_s = big.tile([H, BS], dt, tag="zro")

    nc.sync.dma_start(out=ident_s[:], in_=ident_d[:])
    nc.sync.dma_start(out=wxb_s[:], in_=wxb_d[:])
    for t in range(T):
        nc.vector.tensor_scalar_mul(
            wid_s[:, t * D:(t + 1) * D], ident_s[:], wxb_s[:, t:t + 1])
    nc.vector.memset(zro_s[:], 0.0)

    # x input chunks next (x_score consumes them as they land); the LSTM
    # weights are not needed until the recurrence starts, so they go last.
    for t0 in range(0, T, 8):
        t1 = min(t0 + 8, T)
        nc.sync.dma_start(
            out=xt_s[:, t0 * BL:t1 * BL], in_=xt_d[:, t0:t1, :])
    nc.sync.dma_start(out=rr(wih_s[:]), in_=rr(wih_d[:]))
    nc.sync.dma_start(out=whh_s[:], in_=whh_d[:])
    nc.sync.dma_start(out=rr(bias_s[:]), in_=rr(bias_d[:]))
    nc.sync.dma_start(out=rr(ones_s[:]), in_=rr(ones_d[:]))

    with ExitStack() as fctx:
        fr = fctx.enter_context(tc.tile_pool(name="front", bufs=1))
        frs = fctx.enter_context(tc.tile_pool(name="fsmall", bufs=2))
        psf = fctx.enter_context(tc.tile_pool(name="psf", bufs=1, space="PSUM"))
        pst = fctx.enter_context(tc.tile_pool(name="pstr", bufs=2, space="PSUM"))

        # ---- PE warmup: transpose spins on ident until real work lands ----
        pwm = pst.tile([D, D], dt, tag="warm")
        for w in range(52):
            nc.tensor.transpose(pwm[:], ident_s[:], ident_s[:])

        # ---- x_score in natural [b, d]: ps_xs[j] += (xT_t chunk).T @ wid_t
        # (lhsT = x chunk so the product transposes x back; accumulate over t)
        ps_xs = [psf.tile([BS, D], dt, tag=f"xs{j}", name=f"ps_xs{j}")
                 for j in range(NS)]
        for t in range(T):
            for j in range(NS):
                nc.tensor.matmul(
                    ps_xs[j][:],
                    lhsT=xt_s[:, t * BL + j * BS: t * BL + (j + 1) * BS],
                    rhs=wid_s[:, t * D:(t + 1) * D],
                    start=(t == 0),
                    stop=(t == T - 1),
                )

        # ---- softmax straight off PSUM; transpose attn -> attnT ----
        attnT = big.tile([D, BL], dt, tag="attnT")
        for j in range(NS):
            nmx = frs.tile([BS, 1], dt, tag="nmx")
            nc.vector.tensor_reduce(
                nmx[:], ps_xs[j][:], axis=mybir.AxisListType.X,
                op=mybir.AluOpType.max, negate=True,
            )
            ex = frs.tile([BS, D], dt, tag="ex")
            sums = frs.tile([BS, 1], dt, tag="sums")
            nc.scalar.activation(ex[:], ps_xs[j][:], AF.Exp,
                                 bias=nmx[:], accum_out=sums[:])
            rc = frs.tile([BS, 1], dt, tag="rc")
            nc.vector.reciprocal(rc[:], sums[:])
            at = frs.tile([BS, D], dt, tag="at")
            nc.vector.tensor_scalar_mul(at[:], ex[:], rc[:])

            ptr2 = pst.tile([D, BS], dt, tag="ptr")
            nc.tensor.transpose(ptr2[:], at[:], ident_s[:])
            nc.vector.tensor_copy(attnT[:, j * BS:(j + 1) * BS], ptr2[:])

        # pre-trigger the Sigmoid table-set load while the front finishes
        sdum = frs.tile([BS, 1], dt, tag="sdum")
        nc.scalar.activation(sdum[:], nmx[:], AF.Sigmoid)

    # ---- LSTM recurrence ----
    # PSUM ping-pong tiles [128, 2048]: bank c = gate chunk c (pytorch order
    # i,f,g,o; g pre-scaled 2x), holding [s0_t | s1_t | s0_t+1 | s1_t+1].
    psg = ctx.enter_context(tc.tile_pool(name="psg", bufs=2, space="PSUM"))
    sgp = ctx.enter_context(tc.tile_pool(name="sg", bufs=6))
    sm = ctx.enter_context(tc.tile_pool(name="small", bufs=6))
    hst = ctx.enter_context(tc.tile_pool(name="hstage", bufs=3))
    jk = ctx.enter_context(tc.tile_pool(name="junk", bufs=4))

    c_prev = [zro_s, zro_s]
    h_prev = [zro_s, zro_s]
    hstage = None

    for tg in range(0, T, 2):  # 2-step groups
        gw = min(2, T - tg)                  # steps in this group
        nw = gw * BL                         # bias/W_ih matmul width
        # weighted input for this group: wxT_t = attnT * xT_t (exact fp32
        # for the wt output; fp32r rounded copy for the matmuls)
        for t in range(tg, tg + gw):
            nc.vector.tensor_mul(
                wxt_s[:, t * BL:(t + 1) * BL],
                xt_s[:, t * BL:(t + 1) * BL],
                attnT[:],
            )
            nc.vector.tensor_copy(
                rr(wxr_s[:, t * BL:(t + 1) * BL]),
                wxt_s[:, t * BL:(t + 1) * BL],
            )
        if tg % 8 == 6 or tg == 30:  # flush wt_out every 8 steps
            t0 = (tg // 8) * 8
            t1 = min(t0 + 8, T)
            nc.sync.dma_start(
                out=wt_d[t0:t1].rearrange("t d b -> d t b"),
                in_=wxt_s[:, t0 * BL:t1 * BL].rearrange(
                    "d (t b) -> d t b", b=BL),
            )
        ps = psg.tile([128, 4 * 512], dt, tag="gates")
        # bias + W_ih for both steps of the group, all 4 chunks, N=512
        for c in range(4):
            gseg = slice(c * H, (c + 1) * H)
            nc.tensor.matmul(
                ps[:, c * 512:c * 512 + nw], lhsT=rr(bias_s[0:2, gseg]),
                rhs=rr(ones_s[0:2, 0:nw]), start=True, stop=False,
                skip_group_check=True,
            )
            nc.tensor.matmul(
                ps[:, c * 512:c * 512 + nw], lhsT=rr(wih_s[:, gseg]),
                rhs=rr(wxr_s[:, tg * BL:tg * BL + nw]), start=False, stop=False,
                skip_group_check=True,
            )
        for dtw in range(gw):
            t = tg + dtw
            if t % 4 == 0:
                hstage = hst.tile([H, 4 * BL], dt, tag="hst")
            # phase A: both subtiles' W_hh matmuls + sigmoids, so the ACT
            # queue never has a pointwise-gated op ahead of a ready sigmoid
            sgs = []
            for s in range(NS):
                slot = dtw * 2 + s           # 128-col slot within each bank
                for c in range(4):
                    nc.tensor.matmul(
                        ps[:, c * 512 + slot * BS: c * 512 + (slot + 1) * BS],
                        lhsT=whh_s[:, c * H:(c + 1) * H],
                        rhs=h_prev[s][:],
                        start=False, stop=(slot == 2 * gw - 1),
                        skip_group_check=True,
                    )
                sg = sgp.tile([128, 4 * BS], dt, tag="sg", name=f"sg_{t}_{s}")
                ps_slot = ps[:].rearrange("p (c x) -> p c x", c=4)[
                    :, :, slot * BS:(slot + 1) * BS]
                nc.scalar.activation(sg[:], ps_slot, AF.Sigmoid)
                sgs.append(sg)
            # phase B: pointwise per subtile
            for s in range(NS):
                sg = sgs[s]
                si = sg[:, 0 * BS:1 * BS]
                sf = sg[:, 1 * BS:2 * BS]
                s2g = sg[:, 2 * BS:3 * BS]
                so = sg[:, 3 * BS:4 * BS]

                t1 = sm.tile([H, BS], dt, tag="t1", name=f"t1_{t}_{s}")
                nc.gpsimd.tensor_mul(t1[:], sf, c_prev[s][:])
                t2 = sm.tile([H, BS], dt, tag="t2", name=f"t2_{t}_{s}")
                j1 = jk.tile([H, 1], dt, tag="j1", name=f"j1_{t}_{s}")
                # t2 = tanh(g) * sigmoid(i) = (2*s2g - 1) * si
                nc.vector.affine_mul_reduce(
                    out=t2[:], accum_out=j1[:], in0=s2g, in1=si,
                    scale=2.0, bias=-1.0,
                )
                c_new = sm.tile([H, BS], dt, tag="c", name=f"c_{t}_{s}")
                nc.vector.tensor_add(c_new[:], t1[:], t2[:])
                s2c = sm.tile([H, BS], dt, tag="s2c", name=f"s2c_{t}_{s}")
                nc.scalar.activation(s2c[:], c_new[:], AF.Sigmoid, scale=2.0)
                h_new = hstage[:, (t % 4) * BL + s * BS:
                               (t % 4) * BL + (s + 1) * BS]
                j2 = jk.tile([H, 1], dt, tag="j2", name=f"j2_{t}_{s}")
                # h = tanh(c) * sigmoid(o) = (2*s2c - 1) * so
                nc.vector.affine_mul_reduce(
                    out=h_new, accum_out=j2[:], in0=s2c[:], in1=so,
                    scale=2.0, bias=-1.0,
                )
                c_prev[s] = c_new
                h_prev[s] = _Slice(h_new)
            if t % 4 == 3 or t == T - 1:
                t0 = (t // 4) * 4
                n = t - t0 + 1
                nc.sync.dma_start(
                    out=enc_d[t0:t0 + n].rearrange("t h b -> h t b"),
                    in_=hstage[:].rearrange("h (t b) -> h t b", t=4)[:, :n, :],
                )


class _Slice:
    """Tiny adapter so h_prev[s][:] works for both tiles and AP slices."""

    def __init__(self, ap):
        self._ap = ap

    def __getitem__(self, key):
        return self._ap


def _get_program():
    if "nc" not in _CACHE:
        _CACHE["nc"] = _build_program()
    return _CACHE["nc"]


def _trunc_fp32r(a):
    u = np.ascontiguousarray(a, np.float32).view(np.uint32)
    u = (u + 0x800) & np.uint32(0xFFFFF000)
    return u.view(np.float32)


def _host_inputs(input_data, W_ih, W_hh, b_ih, b_hh, attn_w, attn_b):
    """Build the per-core input maps (host-side prep is weights-only +
    layout transforms)."""
    x = np.ascontiguousarray(input_data, dtype=np.float32)
    W_ih = np.asarray(W_ih, dtype=np.float32)
    W_hh = np.asarray(W_hh, dtype=np.float32)
    b = (np.asarray(b_ih, dtype=np.float32)
         + np.asarray(b_hh, dtype=np.float32))
    w_x = np.asarray(attn_w, dtype=np.float32)[2 * H:]  # only the x-series part

    # scale the g-gate block (pytorch order i,f,g,o -> rows 2H:3H) by 2
    # so tanh(g) = 2*sigmoid(2g) - 1 works with a single sigmoid pass.
    scale = np.ones((G, 1), np.float32)
    scale[2 * H:3 * H] = 2.0
    wih_t = _trunc_fp32r(np.ascontiguousarray((W_ih * scale).T))  # [D, 4H]
    whh_t = np.ascontiguousarray((W_hh * scale).T)          # [H, 4H]
    bm = (b[None, :] * scale.T).astype(np.float32)
    b_hi = _trunc_fp32r(bm)
    b_lo = _trunc_fp32r(bm - b_hi)
    bias_m = np.ascontiguousarray(np.concatenate([b_hi, b_lo], 0))  # [2, 4H]

    wxb = np.ascontiguousarray(np.tile(w_x[None, :], (D, 1)))  # [D, T]
    ident = np.eye(D, dtype=np.float32)
    ones = np.ones((2, 2 * BL), np.float32)

    in_maps = []
    for i in range(NCORES):
        xs = x[i * BL:(i + 1) * BL]                  # [BL, T, D]
        xt = np.ascontiguousarray(xs.transpose(2, 1, 0))  # [D, T, BL]
        in_maps.append({
            "xt": xt,
            "wxb": wxb,
            "wih": wih_t,
            "whh": whh_t,
            "bias": bias_m,
            "ident": ident,
            "ones": ones,
        })
    return in_maps


def _gather(results):
    weighted = np.empty((B, T, D), np.float32)
    encoded = np.empty((B, T, H), np.float32)
    for i, r in enumerate(results):
        # wt_out/enc_out are [T, D|H, BL] -> [BL, T, D|H]
        weighted[i * BL:(i + 1) * BL] = r["wt_out"].transpose(2, 0, 1)
        encoded[i * BL:(i + 1) * BL] = r["enc_out"].transpose(2, 0, 1)
    return weighted, encoded


def kernel(input_data, W_ih, W_hh, b_ih, b_hh, attn_w, attn_b):
    from concourse.bass_utils import run_bass_kernel_spmd

    nc = _get_program()
    in_maps = _host_inputs(input_data, W_ih, W_hh, b_ih, b_hh, attn_w, attn_b)
    res = run_bass_kernel_spmd(nc, in_maps, list(range(NCORES)))
    return _gather(res.results)



# revision 10
# speedup vs baseline: 1.1284x; 1.1284x over previous
"""DA-RNN style encoder (LSTM + input attention) on 8 Trainium2 cores.

Problem: nn_Encoder_63024350101963
  B=2048, T-1=31 steps, D=128 input feats, H=128 hidden.

Key algebraic fact exploited: in the reference,
    score = (h @ w_h + c @ w_c + b)[:, None] + x_score
the recurrent term is constant along the softmax axis, and softmax is
shift-invariant, so
    attn = softmax(x_score)      (time-constant, recurrence-independent)
Therefore weighted = attn[:,None,:] * x  is a pure elementwise op and only
the LSTM cell recurrence is serial.

This version is fp16 end-to-end (tolerance gate is 2e-2):
  - x, weights, and both outputs move as fp16 (halves DMA bytes).
  - all matmuls are fp16 (1 cycle/row at any moving size, vs 4 cyc/row for
    N=128 fp32r at full clock).
  - the LSTM state is cc = 2*c so tanh(c) = 2*sigmoid(cc)-1 with no extra
    scale ops; the g gate is pre-scaled 2x in the weights so a single
    sigmoid pass covers all transcendentals.
  - two independent 128-row batch chains (A/B) per core stay phase-staggered
    so the serial latency of one chain hides under the other's engine work.
  - next group's bias/W_ih matmuls are emitted right after each step's W_hh
    matmuls, so they fill the PE FIFO during the sigmoid/pointwise latency
    window without delaying the recurrence-critical W_hh matmuls.

Sharding: data-parallel over batch, 8 cores x 256 rows, weights replicated.
"""

import numpy as np

T = 31          # time steps (T_ref - 1)
D = 128         # input feature dim
H = 128         # hidden dim
G = 4 * H       # gate rows
NCORES = 8
B = 2048
BL = B // NCORES  # 256 batch rows per core
BS = 128          # batch sub-tile / chain width (2 chains per core)
NS = BL // BS

_CACHE = {}


def _build_program(loop_n=0):
    from contextlib import ExitStack

    import concourse.bacc as bacc
    import concourse.mybir as mybir
    import concourse.tile as tile

    f16 = mybir.dt.float16
    f32 = mybir.dt.float32

    nc = bacc.Bacc("TRN2", target_bir_lowering=False, debug=False)

    xt_d = nc.dram_tensor("xt", [D, T, BL], f16, kind="ExternalInput").ap()
    wxb_d = nc.dram_tensor("wxb", [D, T], f32, kind="ExternalInput").ap()
    wih_d = nc.dram_tensor("wih", [D, G], f16, kind="ExternalInput").ap()
    whh_d = nc.dram_tensor("whh", [H, G], f16, kind="ExternalInput").ap()
    bias_d = nc.dram_tensor("bias", [2, G], f16, kind="ExternalInput").ap()
    ident_d = nc.dram_tensor("ident", [D, D], f16, kind="ExternalInput").ap()
    ones_d = nc.dram_tensor("ones", [2, 2 * BL], f16, kind="ExternalInput").ap()

    wt_d = nc.dram_tensor("wt_out", [T, D, BL], f16, kind="ExternalOutput").ap()
    enc_d = nc.dram_tensor("enc_out", [T, H, BL], f16, kind="ExternalOutput").ap()

    with ExitStack() as ctx:
        tc = ctx.enter_context(tile.TileContext(nc))

        def body():
            _emit(nc, tc, ctx, mybir, f16, f32,
                  xt_d, wxb_d, wih_d, whh_d, bias_d, ident_d, ones_d,
                  wt_d, enc_d)

        if loop_n:
            with tc.For_i(0, loop_n, 1):
                body()
        else:
            body()

    nc.compile()
    return nc


def _emit(nc, tc, ctx, mybir, f16, f32,
          xt_d, wxb_d, wih_d, whh_d, bias_d, ident_d, ones_d, wt_d, enc_d):
    from contextlib import ExitStack

    AF = mybir.ActivationFunctionType

    big = ctx.enter_context(tc.tile_pool(name="big", bufs=1))

    # ---- persistent SBUF tensors ----
    xt_s = big.tile([D, T * BL], f16, tag="xt")
    wxt_s = big.tile([D, T * BL], f16, tag="wxt")
    wid_s = big.tile([D, T * D], f16, tag="wid")
    wxb_s = big.tile([D, T], f32, tag="wxb")
    wih_s = big.tile([D, G], f16, tag="wih")
    whh_s = big.tile([H, G], f16, tag="whh")
    bias_s = big.tile([2, G], f16, tag="bias")
    ident_s = big.tile([D, D], f16, tag="ident")
    ones_s = big.tile([2, 2 * BL], f16, tag="ones")
    zro_s = big.tile([H, BS], f16, tag="zro")
    attnT = big.tile([D, BL], f16, tag="attnT")

    nc.sync.dma_start(out=ident_s[:], in_=ident_d[:])
    nc.sync.dma_start(out=wxb_s[:], in_=wxb_d[:])
    # wid_t = w_x[t] * I  (fp16 scaled identities for the x_score matmuls)
    for t in range(T):
        nc.vector.tensor_scalar_mul(
            wid_s[:, t * D:(t + 1) * D], ident_s[:], wxb_s[:, t:t + 1])
    nc.vector.memset(zro_s[:], 0.0)

    for t0 in range(0, T, 8):
        t1 = min(t0 + 8, T)
        nc.sync.dma_start(
            out=xt_s[:, t0 * BL:t1 * BL], in_=xt_d[:, t0:t1, :])
    nc.sync.dma_start(out=wih_s[:], in_=wih_d[:])
    nc.sync.dma_start(out=whh_s[:], in_=whh_d[:])
    nc.sync.dma_start(out=bias_s[:], in_=bias_d[:])
    nc.sync.dma_start(out=ones_s[:], in_=ones_d[:])

    with ExitStack() as fctx:
        frs = fctx.enter_context(tc.tile_pool(name="fsmall", bufs=2))
        psf = fctx.enter_context(tc.tile_pool(name="psf", bufs=1, space="PSUM"))
        pst = fctx.enter_context(tc.tile_pool(name="pstr", bufs=2, space="PSUM"))

        # ---- PE warmup: transpose spins on ident until real work lands ----
        pwm = pst.tile([D, D], f16, tag="warm")
        for w in range(48):
            nc.tensor.transpose(pwm[:], ident_s[:], ident_s[:])

        # ---- x_score in natural [b, d]: ps_xs[j] += (xT_t chunk).T @ wid_t
        ps_xs = [psf.tile([BS, D], f32, tag=f"xs{j}", name=f"ps_xs{j}")
                 for j in range(NS)]
        for t in range(T):
            for j in range(NS):
                nc.tensor.matmul(
                    ps_xs[j][:],
                    lhsT=xt_s[:, t * BL + j * BS: t * BL + (j + 1) * BS],
                    rhs=wid_s[:, t * D:(t + 1) * D],
                    start=(t == 0),
                    stop=(t == T - 1),
                    skip_group_check=True,
                )

        # ---- softmax straight off PSUM; transpose attn -> attnT ----
        for j in range(NS):
            nmx = frs.tile([BS, 1], f32, tag="nmx")
            nc.vector.tensor_reduce(
                nmx[:], ps_xs[j][:], axis=mybir.AxisListType.X,
                op=mybir.AluOpType.max, negate=True,
            )
            ex = frs.tile([BS, D], f32, tag="ex")
            sums = frs.tile([BS, 1], f32, tag="sums")
            nc.scalar.activation(ex[:], ps_xs[j][:], AF.Exp,
                                 bias=nmx[:], accum_out=sums[:])
            rc = frs.tile([BS, 1], f32, tag="rc")
            nc.vector.reciprocal(rc[:], sums[:])
            at = frs.tile([BS, D], f16, tag="at")
            nc.vector.tensor_scalar_mul(at[:], ex[:], rc[:])

            ptr2 = pst.tile([D, BS], f16, tag="ptr")
            nc.tensor.transpose(ptr2[:], at[:], ident_s[:])
            nc.vector.tensor_copy(attnT[:, j * BS:(j + 1) * BS], ptr2[:])

        # pre-trigger the Sigmoid table-set load while the front finishes
        sdum = frs.tile([BS, 1], f32, tag="sdum")
        nc.scalar.activation(sdum[:], nmx[:], AF.Sigmoid)

    # ---- LSTM recurrence ----
    # PSUM ping-pong tiles [128, 2048] (4 banks); bank c = gate chunk c
    # (pytorch order i,f,g,o; g pre-scaled 2x), cols = [A_t|B_t|A_t+1|B_t+1].
    psg = ctx.enter_context(tc.tile_pool(name="psg", bufs=2, space="PSUM"))
    sgp = ctx.enter_context(tc.tile_pool(name="sg", bufs=6))
    sm = ctx.enter_context(tc.tile_pool(name="small", bufs=8))
    ccp = ctx.enter_context(tc.tile_pool(name="ccp", bufs=6))
    hst = ctx.enter_context(tc.tile_pool(name="hstage", bufs=3))
    jk = ctx.enter_context(tc.tile_pool(name="junk", bufs=4))

    NG = (T + 1) // 2  # 2-step groups

    def emit_group_mms(ps, tg, chunks, piece=BL):
        """bias + W_ih matmuls for the group starting at step tg (cols for
        both chains x both steps of the group).  Emitted in `piece`-wide
        N-slices so they never block a recurrence-critical W_hh matmul in
        the PE queue for more than ~piece cycles."""
        gw = min(2, T - tg)
        nw = gw * BL
        for c in chunks:
            gseg = slice(c * H, (c + 1) * H)
            for n0 in range(0, nw, piece):
                n1 = min(n0 + piece, nw)
                nc.tensor.matmul(
                    ps[:, c * 512 + n0:c * 512 + n1], lhsT=bias_s[0:2, gseg],
                    rhs=ones_s[0:2, n0:n1], start=True, stop=False,
                    skip_group_check=True,
                )
                nc.tensor.matmul(
                    ps[:, c * 512 + n0:c * 512 + n1], lhsT=wih_s[:, gseg],
                    rhs=wxt_s[:, tg * BL + n0:tg * BL + n1],
                    start=False, stop=False,
                    skip_group_check=True,
                )

    def emit_wt(tg):
        """weighted input for group steps (feeds W_ih matmuls + wt output)."""
        for t in range(tg, min(tg + 2, T)):
            nc.vector.tensor_mul(
                wxt_s[:, t * BL:(t + 1) * BL],
                xt_s[:, t * BL:(t + 1) * BL],
                attnT[:],
            )

    c_prev = [zro_s, zro_s]
    h_prev = [zro_s, zro_s]
    hstage = None
    ps_cur = None
    ps_next = None

    # group 0 prep happens up front (overlaps softmax tail / table load)
    emit_wt(0)
    ps_cur = psg.tile([128, 4 * 512], f32, tag="gates", name="ps_g0")
    emit_group_mms(ps_cur, 0, range(4))

    for tg in range(0, T, 2):
        g = tg // 2
        gw = min(2, T - tg)
        if tg + 2 < T:
            emit_wt(tg + 2)
        if tg % 8 == 6 or tg + gw == T:  # flush wt_out every 8 steps
            t0 = (tg // 8) * 8
            t1 = min(t0 + 8, T)
            nc.sync.dma_start(
                out=wt_d[t0:t1].rearrange("t d b -> d t b"),
                in_=wxt_s[:, t0 * BL:t1 * BL].rearrange(
                    "d (t b) -> d t b", b=BL),
            )
        ps_next = None
        for dtw in range(gw):
            t = tg + dtw
            if t % 4 == 0:
                hstage = hst.tile([H, 4 * BL], f16, tag="hst")
            # Emission order is engine-semaphore-aware: each chain's sigmoid
            # follows ITS OWN W_hh matmuls in the PE stream, so its PE-count
            # wait does not cover the other chain's (later-ready) matmuls.
            sgs = []
            for s in range(NS):
                slot = dtw * 2 + s
                for c in range(4):
                    nc.tensor.matmul(
                        ps_cur[:, c * 512 + slot * BS:
                               c * 512 + (slot + 1) * BS],
                        lhsT=whh_s[:, c * H:(c + 1) * H],
                        rhs=h_prev[s][:],
                        start=False, stop=(slot == 2 * gw - 1),
                        skip_group_check=True,
                    )
                sg = sgp.tile([128, 4 * BS], f16, tag="sg", name=f"sg_{t}_{s}")
                ps_slot = ps_cur[:].rearrange("p (c x) -> p c x", c=4)[
                    :, :, slot * BS:(slot + 1) * BS]
                nc.scalar.activation(sg[:], ps_slot, AF.Sigmoid)
                sgs.append(sg)
            # next group's bias/W_ih matmuls go AFTER both sigmoids' deps in
            # the PE stream (no recurrence deps -> PE works ahead on them).
            if tg + 2 < T:
                if ps_next is None:
                    ps_next = psg.tile([128, 4 * 512], f32, tag="gates",
                                       name=f"ps_g{g + 1}")
                emit_group_mms(ps_next, tg + 2, range(2 * dtw, 2 * dtw + 2))
            # pointwise per chain; cc = 2*c state so tanh comes from one
            # sigmoid.  t1 on Pool (off critical path), t2+add fused run on
            # DVE back-to-back (no cross-engine hop on the c path).
            ccs = []
            for s in range(NS):
                sg = sgs[s]
                si = sg[:, 0 * BS:1 * BS]
                sf = sg[:, 1 * BS:2 * BS]
                s2g = sg[:, 2 * BS:3 * BS]

                t1 = sm.tile([H, BS], f16, tag="t1", name=f"t1_{t}_{s}")
                nc.gpsimd.tensor_mul(t1[:], sf, c_prev[s][:])
                t2 = sm.tile([H, BS], f16, tag="t2", name=f"t2_{t}_{s}")
                j1 = jk.tile([H, 1], f32, tag="j1", name=f"j1_{t}_{s}")
                # t2 = 2*sigmoid(i)*tanh(g) = (4*s2g - 2) * si
                nc.vector.affine_mul_reduce(
                    out=t2[:], accum_out=j1[:], in0=s2g, in1=si,
                    scale=4.0, bias=-2.0,
                )
                cc = ccp.tile([H, BS], f16, tag="c", name=f"c_{t}_{s}")
                nc.vector.tensor_add(cc[:], t1[:], t2[:])
                ccs.append(cc)
            for s in range(NS):
                cc = ccs[s]
                so = sgs[s][:, 3 * BS:4 * BS]
                s2c = sm.tile([H, BS], f16, tag="s2c", name=f"s2c_{t}_{s}")
                nc.scalar.activation(s2c[:], cc[:], AF.Sigmoid)
                h_new = hstage[:, (t % 4) * BL + s * BS:
                               (t % 4) * BL + (s + 1) * BS]
                j2 = jk.tile([H, 1], f32, tag="j2", name=f"j2_{t}_{s}")
                # h = tanh(c) * sigmoid(o) = (2*s2c - 1) * so
                nc.vector.affine_mul_reduce(
                    out=h_new, accum_out=j2[:], in0=s2c[:], in1=so,
                    scale=2.0, bias=-1.0,
                )
                c_prev[s] = cc
                h_prev[s] = _Slice(h_new)
            if t % 4 == 3 or t == T - 1:
                t0 = (t // 4) * 4
                n = t - t0 + 1
                nc.sync.dma_start(
                    out=enc_d[t0:t0 + n].rearrange("t h b -> h t b"),
                    in_=hstage[:].rearrange("h (t b) -> h t b", t=4)[:, :n, :],
                )
        ps_cur = ps_next


class _Slice:
    """Tiny adapter so h_prev[s][:] works for both tiles and AP slices."""

    def __init__(self, ap):
        self._ap = ap

    def __getitem__(self, key):
        return self._ap


def _get_program():
    if "nc" not in _CACHE:
        _CACHE["nc"] = _build_program()
    return _CACHE["nc"]


def _host_inputs(input_data, W_ih, W_hh, b_ih, b_hh, attn_w, attn_b):
    """Build the per-core input maps (host-side prep is weights-only +
    layout transforms + fp16 casts)."""
    x = np.asarray(input_data, dtype=np.float32)
    W_ih = np.asarray(W_ih, dtype=np.float32)
    W_hh = np.asarray(W_hh, dtype=np.float32)
    b = (np.asarray(b_ih, dtype=np.float32)
         + np.asarray(b_hh, dtype=np.float32))
    w_x = np.asarray(attn_w, dtype=np.float32)[2 * H:]  # only the x part

    # scale the g-gate block (pytorch order i,f,g,o -> rows 2H:3H) by 2
    # so tanh(g) = 2*sigmoid(2g) - 1 works with a single sigmoid pass.
    scale = np.ones((G, 1), np.float32)
    scale[2 * H:3 * H] = 2.0
    wih_t = np.ascontiguousarray((W_ih * scale).T).astype(np.float16)
    whh_t = np.ascontiguousarray((W_hh * scale).T).astype(np.float16)
    bm = (b[None, :] * scale.T).astype(np.float32)
    b_hi = bm.astype(np.float16)
    b_lo = (bm - b_hi.astype(np.float32)).astype(np.float16)
    bias_m = np.ascontiguousarray(np.concatenate([b_hi, b_lo], 0))  # [2, 4H]

    wxb = np.ascontiguousarray(
        np.tile(w_x[None, :], (D, 1))).astype(np.float32)  # [D, T]
    ident = np.eye(D, dtype=np.float16)
    ones = np.ones((2, 2 * BL), np.float16)

    in_maps = []
    for i in range(NCORES):
        xs = x[i * BL:(i + 1) * BL]                  # [BL, T, D]
        xt = np.ascontiguousarray(
            xs.transpose(2, 1, 0)).astype(np.float16)  # [D, T, BL]
        in_maps.append({
            "xt": xt,
            "wxb": wxb,
            "wih": wih_t,
            "whh": whh_t,
            "bias": bias_m,
            "ident": ident,
            "ones": ones,
        })
    return in_maps


def _gather(results):
    weighted = np.empty((B, T, D), np.float32)
    encoded = np.empty((B, T, H), np.float32)
    for i, r in enumerate(results):
        # wt_out/enc_out are fp16 [T, D|H, BL] -> [BL, T, D|H] fp32
        weighted[i * BL:(i + 1) * BL] = \
            r["wt_out"].transpose(2, 0, 1).astype(np.float32)
        encoded[i * BL:(i + 1) * BL] = \
            r["enc_out"].transpose(2, 0, 1).astype(np.float32)
    return weighted, encoded


def kernel(input_data, W_ih, W_hh, b_ih, b_hh, attn_w, attn_b):
    from concourse.bass_utils import run_bass_kernel_spmd

    nc = _get_program()
    in_maps = _host_inputs(input_data, W_ih, W_hh, b_ih, b_hh, attn_w, attn_b)
    res = run_bass_kernel_spmd(nc, in_maps, list(range(NCORES)))
    return _gather(res.results)
